# revision 1
# baseline (speedup 1.0000x reference)
"""GCN (Linear+ReLU -> GCNConv+ReLU -> GCNConv -> log_softmax) on 8 Trainium2
NeuronCores via Bass.

Sharding: 1D node partition (6250 nodes/core, padded to 6272). Dense GEMMs run
on each core's node slice with activations kept feature-major ("T layout",
features on partitions). The normalized adjacency is factorized as
D^-1/2 (A+I) D^-1/2, so per-edge weights vanish: each layer scales its
projected features by dinv once (the gather table g = dinv * (h @ W)), the
edge aggregation is a plain unweighted segment sum, and the destination scale
dinv[d] is applied on the way out of PSUM.

Aggregation: the projected/scaled feature table is all-gathered (bf16,
row-padded to 256 B), then each core gathers its in-edges' source rows with
per-edge DMA-gather descriptors. Destinations are packed into 128-node
"windows" sorted by in-degree so the segment sum becomes ELL-style rounds:
each round is one [128 x 128] tile whose partition p belongs to window
position p, accumulated into PSUM with an identity-stationary matmul. Pad
slots point at an all-zero table row (a padded node column), so no masking is
needed anywhere.
"""

import os
import sys
from contextlib import ExitStack
from dataclasses import dataclass, field

import numpy as np

sys.path.insert(0, "/opt/trn_rl_repo")

import ml_dtypes  # noqa: E402

BF16 = ml_dtypes.bfloat16

# ---------------------------------------------------------------- config


@dataclass
class Cfg:
    N: int = 50000
    E: int = 800000
    FIN: int = 500
    H1: int = 300
    H2: int = 100
    C: int = 16
    NCORES: int = 8

    FP: int = 512      # padded FIN (contraction tiles of 128)
    H2P: int = 128     # padded H2
    F1C: int = 100     # H1 chunk width (3 chunks of 100)

    NCR: int = field(init=False)   # real nodes per core
    NCP: int = field(init=False)   # padded nodes per core (x128)
    NW: int = field(init=False)    # windows per core
    NTOT: int = field(init=False)  # padded global table rows
    HALF: int = field(init=False)  # table half size (int16 index reach)

    def __post_init__(self):
        assert self.N % self.NCORES == 0
        self.NCR = self.N // self.NCORES
        self.NCP = ((self.NCR + 127) // 128) * 128
        assert self.NCP > self.NCR, "need at least one pad column per core"
        self.NW = self.NCP // 128
        self.NTOT = self.NCP * self.NCORES
        assert self.NTOT % 2 == 0
        self.HALF = self.NTOT // 2
        assert self.HALF <= 32767, "table half must be int16-indexable"
        assert self.H1 % self.F1C == 0


FULL = Cfg()

# ---------------------------------------------------------------- host prep


@dataclass
class Meta:
    """Compile-time structure shared by all cores (SPMD)."""
    RA: np.ndarray
    RB: np.ndarray
    calls: list          # (slot_off, n_slots, half)
    win_chunks: list     # per window: [(call_idx, row_in_call), ...]
    SLOTS: int


def prep_graph(cfg: Cfg, edge_index: np.ndarray):
    """Host-side index preprocessing: sharding, window packing, slot arrays."""
    src = edge_index[0].astype(np.int64)
    dst = edge_index[1].astype(np.int64)
    deg = np.bincount(dst, minlength=cfg.N).astype(np.float64) + 1.0
    dinv = (1.0 / np.sqrt(deg)).astype(np.float32)

    score = src // cfg.NCR
    sloc = src % cfg.NCR
    dcore = dst // cfg.NCR
    dloc = dst % cfg.NCR

    # pass 1: per-core destination window packing (sorted by in-degree per half)
    cores = []
    # src half needs each core's qpos; compute A/B by *global padded row*,
    # which itself needs qpos of the src's core -> do half split after pass 1.
    for c in range(cfg.NCORES):
        m = dcore == c
        cores.append(dict(mask=m, dl=dloc[m], s=src[m]))

    # temporary halves based on raw source core (first 4 cores -> half A)
    # (global row = score*NCP + qpos[sloc]; half = score >= NCORES//2 exactly,
    #  since rows are core-major: core c occupies [c*NCP, (c+1)*NCP).)
    half_of_src = (score >= cfg.NCORES // 2).astype(np.int64)

    for c in range(cfg.NCORES):
        cc = cores[c]
        h = half_of_src[cc["mask"]]
        dA = np.bincount(cc["dl"][h == 0], minlength=cfg.NCP)
        dB = np.bincount(cc["dl"][h == 1], minlength=cfg.NCP)
        order = np.lexsort((-dB, -dA))  # primary dA desc, then dB desc
        qpos = np.empty(cfg.NCP, np.int64)
        qpos[order] = np.arange(cfg.NCP)
        RA = np.zeros(cfg.NW, np.int64)
        RB = np.zeros(cfg.NW, np.int64)
        for w in range(cfg.NW):
            sel = order[w * 128:(w + 1) * 128]
            RA[w] = dA[sel].max()
            RB[w] = dB[sel].max()
        cc.update(h=h, dA=dA, dB=dB, order=order, qpos=qpos, RA=RA, RB=RB)

    RA = np.maximum.reduce([cc["RA"] for cc in cores])
    RB = np.maximum.reduce([cc["RB"] for cc in cores])
    for w in range(cfg.NW):
        if RA[w] + RB[w] == 0:
            RA[w] = 1

    # global slot layout: groups of 4 windows, A span then B span
    groups = [list(range(i, min(i + 4, cfg.NW))) for i in range(0, cfg.NW, 4)]
    calls = []
    win_chunks = [[] for _ in range(cfg.NW)]
    baseA = np.zeros(cfg.NW, np.int64)
    baseB = np.zeros(cfg.NW, np.int64)
    off = 0
    for g in groups:
        for hh, (RR, base) in enumerate(((RA, baseA), (RB, baseB))):
            rows = 0
            ci = len(calls)
            for w in g:
                base[w] = off + rows * 128
                for r in range(int(RR[w])):
                    win_chunks[w].append((ci, rows + r))
                rows += int(RR[w])
            if rows:
                calls.append((off, rows * 128, hh))
                off += rows * 128
    SLOTS = off
    assert SLOTS % 16 == 0
    meta = Meta(RA=RA, RB=RB, calls=calls, win_chunks=win_chunks, SLOTS=SLOTS)

    # pass 2: fill per-core slot index arrays
    # global (padded, device-ordered) table row of each edge's source:
    qpos_all = np.concatenate([cc["qpos"] for cc in cores])  # index by core*NCP + loc
    grow = score * cfg.NCP + qpos_all[score * cfg.NCP + sloc]

    zrowA = int(cores[0]["qpos"][cfg.NCR])  # core0 pad column: all-zero row
    c4 = cfg.NCORES // 2
    zrowB = int(cores[c4]["qpos"][cfg.NCR]) + c4 * cfg.NCP - cfg.HALF
    assert 0 <= zrowA < cfg.HALF and 0 <= zrowB < cfg.HALF

    for c in range(cfg.NCORES):
        cc = cores[c]
        m = cc["mask"]
        eg = grow[m]
        eh = cc["h"]
        edl = cc["dl"]
        assert np.all((eg >= cfg.HALF) == (eh == 1))
        ew = cc["qpos"][edl] // 128
        ep = cc["qpos"][edl] % 128
        # rank of edge within its (dloc, half) group
        key = edl * 2 + eh
        o = np.argsort(key, kind="stable")
        ks = key[o]
        first = np.r_[0, np.flatnonzero(ks[1:] != ks[:-1]) + 1]
        starts = np.zeros(len(ks), np.int64)
        starts[first] = first
        starts = np.maximum.accumulate(starts)
        rank = np.empty(len(ks), np.int64)
        rank[o] = np.arange(len(ks)) - starts

        idxv = np.full(SLOTS, -1, np.int64)
        for coff, n, hh in calls:
            idxv[coff:coff + n] = zrowA if hh == 0 else zrowB
        posA = baseA[ew] + rank * 128 + ep
        posB = baseB[ew] + rank * 128 + ep
        pos = np.where(eh == 0, posA, posB)
        val = np.where(eh == 0, eg, eg - cfg.HALF)
        idxv[pos] = val
        assert idxv.min() >= 0 and idxv.max() < cfg.HALF
        gi = idxv.reshape(-1, 16).T.astype(np.int16)         # [16, SLOTS/16]
        cc["gidx"] = np.tile(gi, (8, 1))                     # [128, SLOTS/16]
    return dinv, cores, meta


def prep_inputs(cfg: Cfg, inputs: dict, dinv, cores, meta: Meta):
    """Build per-core in_maps (numpy) for the device kernel."""
    x = np.asarray(inputs["x"], np.float32)
    lin_W = np.asarray(inputs["lin_W"], np.float32)
    lin_b = np.asarray(inputs["lin_b"], np.float32)
    W1 = np.asarray(inputs["W1"], np.float32)
    b1 = np.asarray(inputs["b1"], np.float32)
    W2 = np.asarray(inputs["W2"], np.float32)
    b2 = np.asarray(inputs["b2"], np.float32)

    linWp = np.zeros((cfg.FP, cfg.H1), BF16)
    linWp[:cfg.FIN] = lin_W.astype(BF16)
    nf1 = cfg.H1 // cfg.F1C
    linbp = lin_b.reshape(nf1, cfg.F1C).T.astype(np.float32).copy()  # [F1C, nf1]
    W1p = np.zeros((cfg.H1, cfg.H2P), BF16)
    W1p[:, :cfg.H2] = W1.astype(BF16)
    W2p = np.zeros((cfg.H2P, cfg.C), BF16)
    W2p[:cfg.H2] = W2.astype(BF16)
    b1rep = np.zeros((128, cfg.H2P), np.float32)
    b1rep[:, :cfg.H2] = b1
    b2rep = np.tile(b2.reshape(1, cfg.C), (128, 1)).astype(np.float32)
    ident = np.eye(128, dtype=BF16)

    xT = np.zeros((cfg.FP, cfg.N), np.float32)
    xT[:cfg.FIN] = x.T

    in_maps = []
    for c in range(cfg.NCORES):
        cc = cores[c]
        order = cc["order"]
        real = order < cfg.NCR
        gcols = np.where(real, cfg.NCR * c + np.minimum(order, cfg.NCR - 1), 0)
        xTc = xT[:, gcols] * real[None, :]
        dv = dinv[gcols] * real
        dinvT = np.tile(dv.astype(BF16).reshape(1, -1), (128, 1))
        dinvN = dv.reshape(cfg.NW, 128).T.astype(np.float32).copy()
        in_maps.append({
            "xT": xTc.astype(BF16),
            "linW": linWp, "linb": linbp,
            "W1": W1p, "W2": W2p,
            "b1rep": b1rep, "b2rep": b2rep,
            "dinvT": dinvT, "dinvN": dinvN,
            "ident": ident, "gidx": cc["gidx"],
        })
    return in_maps


def assemble_output(cfg: Cfg, cores, outs):
    """outs: per-core [128, NW*C] -> full [N, C] float32."""
    res = np.empty((cfg.N, cfg.C), np.float32)
    for c in range(cfg.NCORES):
        o = np.asarray(outs[c]).reshape(128, cfg.NW, cfg.C)
        o = o.transpose(1, 0, 2).reshape(cfg.NCP, cfg.C)  # device node order
        order = cores[c]["order"]
        real = order < cfg.NCR
        res[c * cfg.NCR + order[real]] = o[real]
    return res


# ---------------------------------------------------------------- device kernel


def build_nc(cfg: Cfg, meta: Meta, stage: int = 9):
    import concourse.bacc as bacc
    import concourse.mybir as mybir
    import concourse.tile as tile

    dt = mybir.dt
    f32, bf16, i16 = dt.float32, dt.bfloat16, dt.int16
    AF = mybir.ActivationFunctionType
    OP = mybir.AluOpType

    nc = bacc.Bacc("TRN2", target_bir_lowering=False, debug=False,
                   enable_asserts=False, num_devices=cfg.NCORES,
                   num_swdge_queues=2)

    NCP, NW, NTOT, HALF, C = cfg.NCP, cfg.NW, cfg.NTOT, cfg.HALF, cfg.C
    F1C, H2P = cfg.F1C, cfg.H2P
    NK = cfg.FP // 128          # contraction tiles for GEMM1
    NF1 = cfg.H1 // F1C         # feature chunks of h1

    xT_d = nc.dram_tensor("xT", [cfg.FP, NCP], bf16, kind="ExternalInput")
    linW_d = nc.dram_tensor("linW", [cfg.FP, cfg.H1], bf16, kind="ExternalInput")
    linb_d = nc.dram_tensor("linb", [F1C, NF1], f32, kind="ExternalInput")
    W1_d = nc.dram_tensor("W1", [cfg.H1, H2P], bf16, kind="ExternalInput")
    W2_d = nc.dram_tensor("W2", [H2P, C], bf16, kind="ExternalInput")
    b1r_d = nc.dram_tensor("b1rep", [128, H2P], f32, kind="ExternalInput")
    b2r_d = nc.dram_tensor("b2rep", [128, C], f32, kind="ExternalInput")
    dvT_d = nc.dram_tensor("dinvT", [128, NCP], bf16, kind="ExternalInput")
    dvN_d = nc.dram_tensor("dinvN", [128, NW], f32, kind="ExternalInput")
    id_d = nc.dram_tensor("ident", [128, 128], bf16, kind="ExternalInput")
    gi_d = nc.dram_tensor("gidx", [128, meta.SLOTS // 16], i16, kind="ExternalInput")
    out_d = nc.dram_tensor("out", [128, NW * C], f32, kind="ExternalOutput")

    GW = [(i, min(512, NCP - i)) for i in range(0, NCP, 512)]
    groups_of = {}
    for ci, (coff, n, hh) in enumerate(meta.calls):
        groups_of[ci] = (coff, n, hh)

    with tile.TileContext(nc) as tc, ExitStack() as top:
        const = top.enter_context(tc.tile_pool(name="const", bufs=1))
        dram = top.enter_context(tc.tile_pool(name="dram", bufs=1, space="DRAM"))

        ident = const.tile([128, 128], bf16)
        nc.sync.dma_start(ident[:], id_d[:])
        dinvT = const.tile([128, NCP], bf16)
        nc.sync.dma_start(dinvT[:], dvT_d[:])
        dinvN = const.tile([128, NW], f32)
        nc.sync.dma_start(dinvN[:], dvN_d[:])
        b1rep = const.tile([128, H2P], f32)
        nc.sync.dma_start(b1rep[:], b1r_d[:])
        b2rep = const.tile([128, C], f32)
        nc.sync.dma_start(b2rep[:], b2r_d[:])
        linb = const.tile([F1C, NF1], f32)
        nc.sync.dma_start(linb[:], linb_d[:])
        gidx = const.tile([128, meta.SLOTS // 16], i16)
        nc.sync.dma_start(gidx[:], gi_d[:])
        W1t = []
        for f in range(NF1):
            t = const.tile([F1C, H2P], bf16, name=f"W1t{f}")
            nc.sync.dma_start(t[:], W1_d[f * F1C:(f + 1) * F1C, :])
            W1t.append(t)
        W2t = const.tile([H2P, C], bf16)
        nc.sync.dma_start(W2t[:], W2_d[:])

        g1T = const.tile([128, NCP], bf16, tag="bigA")
        g1nat = const.tile([128, NW, 128], bf16, tag="bigB")
        h2nat = const.tile([128, NW, H2P], bf16)
        h2T = const.tile([128, NW, 128], bf16, tag="bigA")  # [f, w, p]
        g2Tf = const.tile([128, NCP], bf16)
        g2nat = const.tile([128, NW, 128], bf16, tag="bigB")
        logit = const.tile([128, NW, C], f32)
        outsb = const.tile([128, NW, C], f32)

        g1loc = dram.tile([NCP, 128], bf16)
        g2loc = dram.tile([NCP, 128], bf16)
        full1 = dram.tile([NTOT, 128], bf16, addr_space="Shared")
        full2 = dram.tile([NTOT, 128], bf16, addr_space="Shared")

        # ---- phase A: GEMM1 (relu(x@linW+b)) and GEMM2 (g1 = dinv*(h1@W1))
        with ExitStack() as ph:
            xp = ph.enter_context(tc.tile_pool(name="xp", bufs=1))
            hp = ph.enter_context(tc.tile_pool(name="hp", bufs=1))
            ps = ph.enter_context(tc.tile_pool(name="psA", bufs=4,
                                               space="PSUM"))
            lw = ph.enter_context(tc.tile_pool(name="lw", bufs=1))
            xt = []
            for k in range(NK):
                t = xp.tile([128, NCP], bf16, name=f"xt{k}")
                nc.sync.dma_start(t[:], xT_d[k * 128:(k + 1) * 128, :])
                xt.append(t)
            lwt = []
            for k in range(NK):
                t = lw.tile([128, cfg.H1], bf16, name=f"lwt{k}")
                nc.sync.dma_start(t[:], linW_d[k * 128:(k + 1) * 128, :])
                lwt.append(t)
            h1 = []
            for f in range(NF1):
                h1.append(hp.tile([F1C, NCP], bf16, name=f"h1{f}"))
            for f in range(NF1):
                for (c0, cw) in GW:
                    acc = ps.tile([F1C, 512], f32, tag="accA")
                    for k in range(NK):
                        nc.tensor.matmul(
                            acc[:, :cw],
                            lwt[k][:, f * F1C:(f + 1) * F1C],
                            xt[k][:, c0:c0 + cw],
                            start=(k == 0), stop=(k == NK - 1))
                    nc.scalar.activation(h1[f][:, c0:c0 + cw], acc[:, :cw],
                                         AF.Relu, bias=linb[:, f:f + 1])
            for (c0, cw) in GW:
                acc = ps.tile([H2P, 512], f32, tag="accB")
                for f in range(NF1):
                    nc.tensor.matmul(acc[:, :cw], W1t[f][:],
                                     h1[f][:, c0:c0 + cw],
                                     start=(f == 0), stop=(f == NF1 - 1))
                nc.vector.tensor_mul(g1T[:, c0:c0 + cw], acc[:, :cw],
                                     dinvT[:, c0:c0 + cw])

        # table -> natural layout -> local DRAM -> AllGather
        if stage >= 2 or stage in (31, 32):
            nc.sync.dma_start_transpose(g1nat[:], g1T[:])
            nc.sync.dma_start(g1loc[:].rearrange("(w p) f -> p w f", p=128),
                              g1nat[:])
            nc.gpsimd.collective_compute(
                "AllGather", OP.bypass,
                replica_groups=[list(range(cfg.NCORES))],
                ins=[g1loc[:]], outs=[full1[:]])

        # ---- scatter layers
        def scatter(full, gnat, layer, mode=3):
            with ExitStack() as ph:
                gp = ph.enter_context(tc.tile_pool(name=f"gb{layer}", bufs=6))
                pp = ph.enter_context(tc.tile_pool(name=f"psW{layer}", bufs=6,
                                                   space="PSUM"))
                ep = ph.enter_context(tc.tile_pool(name=f"ep{layer}", bufs=4))
                gtiles = {}
                maxrows = max(n // 128 for _, n, _ in meta.calls)
                done_w = 0
                for ci, (coff, n, hh) in enumerate(meta.calls):
                    t = gp.tile([128, maxrows, 128], bf16, tag="gb")
                    for si, s0 in enumerate(range(0, n, 1024)):
                        sn = min(1024, n - s0)
                        nc.gpsimd.dma_gather(
                            t[:, s0 // 128:(s0 + sn) // 128, :],
                            full[hh * HALF:(hh + 1) * HALF, :],
                            gidx[:, (coff + s0) // 16:(coff + s0 + sn) // 16],
                            num_idxs=sn, num_idxs_reg=sn, elem_size=128,
                            queue_num=(ci + si) % 2)
                    gtiles[ci] = t
                    # emit windows whose chunks are all gathered
                    while done_w < NW and all(
                            c <= ci for c, _ in meta.win_chunks[done_w]):
                        w = done_w
                        chunks = meta.win_chunks[w]
                        if mode == 1:
                            done_w += 1
                            continue
                        if layer == 1:
                            acc = pp.tile([128, 128], f32, tag="pw")
                            for k, (cidx, row) in enumerate(chunks):
                                nc.tensor.matmul(
                                    acc[:], ident[:],
                                    gtiles[cidx][:, row, :],
                                    start=(k == 0), stop=(k == len(chunks) - 1))
                            if mode == 2:
                                nc.vector.tensor_copy(h2nat[:, w, :], acc[:])
                            else:
                                t1 = ep.tile([128, H2P], f32, tag="t1")
                                nc.vector.scalar_tensor_tensor(
                                    t1[:], acc[:], 0.0, gnat[:, w, :],
                                    OP.add, OP.add)
                                t2 = ep.tile([128, H2P], f32, tag="t2")
                                nc.vector.scalar_tensor_tensor(
                                    t2[:], t1[:], dinvN[:, w:w + 1], b1rep[:],
                                    OP.mult, OP.add)
                                nc.scalar.activation(h2nat[:, w, :], t2[:], AF.Relu)
                        else:
                            acc = pp.tile([128, C], f32, tag="pw")
                            for k, (cidx, row) in enumerate(chunks):
                                nc.tensor.matmul(
                                    acc[:], ident[:],
                                    gtiles[cidx][:, row, :C],
                                    start=(k == 0), stop=(k == len(chunks) - 1))
                            t1 = ep.tile([128, C], f32, tag="t1")
                            nc.vector.scalar_tensor_tensor(
                                t1[:], acc[:], 0.0, gnat[:, w, :C],
                                OP.add, OP.add)
                            nc.vector.scalar_tensor_tensor(
                                logit[:, w, :], t1[:], dinvN[:, w:w + 1],
                                b2rep[:], OP.mult, OP.add)
                        done_w += 1
                assert done_w == NW

        if stage >= 3:
            scatter(full1, g1nat, layer=1,
                    mode=(1 if stage == 31 else 2 if stage == 32 else 3))

        # h2 natural -> T layout; GEMM3; g2 table; AllGather
        if stage >= 4 and stage not in (31, 32):
            nc.sync.dma_start_transpose(h2T[:], h2nat[:].rearrange("p w f -> p (w f)"))
            nc.gpsimd.memset(g2Tf[:], 0.0)
            with ExitStack() as ph:
                ps3 = ph.enter_context(tc.tile_pool(name="ps3", bufs=2, space="PSUM"))
                h2Tf = h2T[:].rearrange("f w p -> f (w p)")
                for (c0, cw) in GW:
                    acc = ps3.tile([C, 512], f32, tag="acc3")
                    nc.tensor.matmul(acc[:, :cw], W2t[:], h2Tf[:, c0:c0 + cw],
                                     start=True, stop=True)
                    nc.vector.tensor_mul(g2Tf[:C, c0:c0 + cw], acc[:, :cw],
                                         dinvT[:C, c0:c0 + cw])
            nc.sync.dma_start_transpose(g2nat[:], g2Tf[:])
            nc.sync.dma_start(g2loc[:].rearrange("(w p) f -> p w f", p=128),
                              g2nat[:])
            nc.gpsimd.collective_compute(
                "AllGather", OP.bypass,
                replica_groups=[list(range(cfg.NCORES))],
                ins=[g2loc[:]], outs=[full2[:]])
        if stage >= 5 and stage not in (31, 32):
            scatter(full2, g2nat, layer=2)

        nc._dbg = dict(g1loc=g1loc, g2loc=g2loc, full1=full1, full2=full2,
                       g1T=g1T, h2nat=h2nat, logit=logit, g1nat=g1nat)

        # ---- log_softmax over C (free dim), no max-subtraction (logits small)
        if stage < 6 or stage in (31, 32):
            nc.gpsimd.memset(logit[:], 0.0)
        with ExitStack() as ph:
            sp = ph.enter_context(tc.tile_pool(name="sm", bufs=1))
            et = sp.tile([128, NW, C], f32)
            nc.scalar.activation(et[:], logit[:], AF.Exp)
            ssum = sp.tile([128, NW], f32)
            nc.vector.tensor_reduce(ssum[:], et[:], mybir.AxisListType.X,
                                    OP.add)
            negl = sp.tile([128, NW], f32)
            nc.scalar.activation(negl[:], ssum[:], AF.Ln)
            nc.vector.tensor_scalar_mul(negl[:], negl[:], -1.0)
            for w in range(NW):
                nc.scalar.activation(outsb[:, w, :], logit[:, w, :],
                                     AF.Identity, bias=negl[:, w:w + 1])
            nc.sync.dma_start(out_d[:].rearrange("p (w c) -> p w c", c=C),
                              outsb[:])

    nc.compile()
    return nc


# ---------------------------------------------------------------- entry

_CACHE = {}


def _get_nc(cfg: Cfg, meta: Meta):
    key = (cfg.N, cfg.E, meta.SLOTS, tuple(meta.RA), tuple(meta.RB))
    if key not in _CACHE:
        _CACHE[key] = build_nc(cfg, meta)
    return _CACHE[key]


def run(cfg: Cfg, inputs: dict, trace: bool = False):
    from concourse.bass_utils import run_bass_kernel_spmd
    dinv, cores, meta = prep_graph(cfg, np.asarray(inputs["edge_index"]))
    in_maps = prep_inputs(cfg, inputs, dinv, cores, meta)
    nc = _get_nc(cfg, meta)
    try:
        res = run_bass_kernel_spmd(nc, in_maps,
                                   core_ids=list(range(cfg.NCORES)),
                                   trace=trace)
    except ModuleNotFoundError:
        res = run_bass_kernel_spmd(nc, in_maps,
                                   core_ids=list(range(cfg.NCORES)),
                                   trace=False)
    out = assemble_output(cfg, cores, [r["out"] for r in res.results])
    return out, res


def kernel(**inputs) -> np.ndarray:
    out, _ = run(FULL, inputs)
    return out


def bench_chain(cfg: Cfg, inputs: dict, iters: int = 8):
    """Time device execution by chaining `iters` NEFF executions in one jit
    (output of run k feeds the donated output buffer of run k+1, serializing
    them); returns (per_exec_seconds, outputs_of_last_run)."""
    import time as _time

    import jax
    import numpy as _np
    from jax.experimental.shard_map import shard_map
    from jax.sharding import Mesh, PartitionSpec

    import concourse.mybir as mybir
    from concourse import bass2jax

    dinv, cores, meta = prep_graph(cfg, np.asarray(inputs["edge_index"]))
    in_maps = prep_inputs(cfg, inputs, dinv, cores, meta)
    nc = _get_nc(cfg, meta)
    bass2jax.install_neuronx_cc_hook()

    pname = nc.partition_id_tensor.name if nc.partition_id_tensor else None
    in_names, out_names, out_avals, zero_outs = [], [], [], []
    for alloc in nc.m.functions[0].allocations:
        if not isinstance(alloc, mybir.MemoryLocationSet):
            continue
        name = alloc.memorylocations[0].name
        if alloc.kind == "ExternalInput":
            if name != pname:
                in_names.append(name)
        elif alloc.kind == "ExternalOutput":
            out_names.append(name)
            shape = tuple(alloc.tensor_shape)
            dtype = mybir.dt.np(alloc.dtype)
            out_avals.append(jax.core.ShapedArray(shape, dtype))
            zero_outs.append(_np.zeros(shape, dtype))
    n_params = len(in_names)
    all_names = in_names + out_names + ([pname] if pname else [])

    def _body_n(n_execs, *args):
        operands = list(args[:n_params])
        outs = list(args[n_params:])
        pid = [bass2jax.partition_id_tensor()] if pname else []
        for _ in range(n_execs):
            outs = list(bass2jax._bass_exec_p.bind(
                *operands, *outs, *pid,
                out_avals=tuple(out_avals),
                in_names=tuple(all_names),
                out_names=tuple(out_names),
                lowering_input_output_aliases=(),
                sim_require_finite=True, sim_require_nnan=True, nc=nc))
        return tuple(outs)

    devices = jax.devices()[:cfg.NCORES]
    mesh = Mesh(_np.asarray(devices), ("core",))
    spec = (PartitionSpec("core"),)
    concat_in = [_np.concatenate([_np.asarray(in_maps[c][n])
                                  for c in range(cfg.NCORES)], axis=0)
                 for n in in_names]
    concat_zeros = [_np.zeros((cfg.NCORES * z.shape[0], *z.shape[1:]), z.dtype)
                    for z in zero_outs]
    nin = n_params + len(zero_outs)
    fn = jax.jit(
        shard_map(lambda *a: _body_n(1, *a), mesh=mesh,
                  in_specs=spec * nin, out_specs=spec * len(out_names),
                  check_rep=False),
        donate_argnums=tuple(range(n_params, nin)), keep_unused=True)
    din = [jax.device_put(x) for x in concat_in]
    outs = fn(*din, *concat_zeros)  # compile+warm
    jax.block_until_ready(outs)
    # async pipeline: issue `iters` executions back-to-back, block once.
    zzs = []
    for _ in range(iters):
        zzs.append([jax.device_put(
            _np.zeros((cfg.NCORES * z.shape[0], *z.shape[1:]), z.dtype))
            for z in zero_outs])
    jax.block_until_ready(zzs)
    t0 = _time.perf_counter()
    all_outs = [fn(*din, *zz) for zz in zzs]
    jax.block_until_ready(all_outs)
    t_pipe = _time.perf_counter() - t0
    times = [t_pipe / iters]
    per_exec = times[0]
    last = all_outs[-1]
    outs_np = [_np.asarray(last[i]).reshape(cfg.NCORES, *out_avals[i].shape)
               for i in range(len(out_names))]
    out = assemble_output(cfg, cores,
                          [outs_np[out_names.index("out")][c]
                           for c in range(cfg.NCORES)])
    return per_exec, times, out

